# revision 14
# baseline (speedup 1.0000x reference)
"""Trainium2 Bass kernel for DynamicEdgeConstruction (top-k masked softmax
attention matrix).

Computes, for x [B=4, N=4096, C=256], W_q/W_k [256, 64]:
    Q = x @ W_q; K = x @ W_k
    S = Q K^T / sqrt(64)           [B, N, N]
    A = softmax over the top-k entries of each row of S, zeros elsewhere.

Sharding: 8 NeuronCores, 2 per batch element, each handling 2048 query rows
(row-wise sequence parallel; K replicated per batch).

Device algorithm per core (candidate-group formulation): the dense A is
~0.2% nonzero, so the device never materializes it.  Per 128-row tile it
computes S via bf16 PE matmuls (fp32 PSUM) and folds the 4096 columns
through a 3-level max-reduction tree into 512 fp16 group-maxima (groups =
columns congruent mod 512), which it ships to the host.  Every true top-k
column's group provably ranks <= 8 among the 512 group-maxima up to
bf16/fp16 rounding, so the host's top-24 group pick has large slack.  The
host gathers the 24x8 candidate columns, recomputes their exact fp32
scores (Q/K host-side), picks the exact top-k with lax.top_k tie
semantics, and scatters the softmax values into the dense fp32 output.

Engine split per tile (ns, cost-model): PE 8 matmuls ~1.7-3.4k; ACT casts
pb + pa-tail to fp16 SBUF ~3.2k; DVE L1 (ZPS cols straight from PSUM) +
L2 + L3 ~3.2k; DMA ships 1 KiB/row of group-maxima.
"""

import numpy as np

B, N, C, DK = 4, 4096, 256, 64
NCORES = 8
RPC = N // 2          # rows per core (2048)
P = 128               # partitions
NT = RPC // P         # row tiles per core (16)
CHUNK = 512           # matmul free-dim chunk (one PSUM bank fp32)
HALF = 2048
NGROUP = 512          # leaf groups per row (columns congruent mod 512)
ZPS = 1216            # L1 columns DVE takes straight from PSUM (rest via ACT)
NCAND = 24            # groups the host keeps per row (top-24 of 512)

_cache = {}


def _build():
    import concourse.bass as bass
    import concourse.bacc as bacc
    import concourse.tile as tile
    import concourse.mybir as mybir
    from contextlib import ExitStack

    f32 = mybir.dt.float32
    f16 = mybir.dt.float16
    bf16 = mybir.dt.bfloat16
    u16 = mybir.dt.uint16
    mx = mybir.AluOpType.max

    nc = bacc.Bacc("TRN2", target_bir_lowering=False, debug=False,
                   num_devices=NCORES)

    xk_d = nc.dram_tensor("xk", [C, N], bf16, kind="ExternalInput").ap()
    wq_d = nc.dram_tensor("wq", [C, DK], bf16, kind="ExternalInput").ap()
    wk_d = nc.dram_tensor("wk", [C, DK], bf16, kind="ExternalInput").ap()
    d3_d = nc.dram_tensor("d3", [RPC, NGROUP], f16, kind="ExternalOutput").ap()

    with tile.TileContext(nc) as tc:
        with ExitStack() as ctx:
            const = ctx.enter_context(tc.tile_pool(name="const", bufs=1))

            xk = [const.tile([P, N], bf16, tag=f"xk{i}", name=f"xk{i}")
                  for i in range(2)]
            wq = [const.tile([P, DK], bf16, tag=f"wq{i}", name=f"wq{i}")
                  for i in range(2)]
            wk = [const.tile([P, DK], bf16, tag=f"wk{i}", name=f"wk{i}")
                  for i in range(2)]
            KT = const.tile([DK, N], bf16, tag="KT")
            QT = const.tile([DK, RPC], bf16, tag="QT")

            # weights first (small), then x in column chunks on two queues
            nc.gpsimd.dma_start(wk[0][:], wk_d[0:P, :])
            nc.gpsimd.dma_start(wk[1][:], wk_d[P:2 * P, :])
            nc.gpsimd.dma_start(wq[0][:], wq_d[0:P, :])
            nc.gpsimd.dma_start(wq[1][:], wq_d[P:2 * P, :])
            XCH = 1024
            for chx in range(N // XCH):
                sl = slice(chx * XCH, (chx + 1) * XCH)
                nc.sync.dma_start(xk[0][:, sl], xk_d[0:P, sl])
                nc.scalar.dma_start(xk[1][:, sl], xk_d[P:2 * P, sl])

            # Projections: KT = wk^T @ x (contraction over C in two halves),
            # QT from columns [0, RPC) -- the host rotates x per core so this
            # core's query columns always sit first, and un-rotates the
            # candidate column ids afterwards.
            with tc.tile_pool(name="proj_ps", bufs=2, space="PSUM") as proj_ps:
                def proj_into(dst, w, sl):
                    pt = proj_ps.tile([DK, CHUNK], f32, tag="proj", name="pt")
                    nc.tensor.matmul(out=pt[:], lhsT=w[0][:],
                                     rhs=xk[0][:, sl], start=True, stop=False)
                    nc.tensor.matmul(out=pt[:], lhsT=w[1][:],
                                     rhs=xk[1][:, sl], start=False, stop=True)
                    nc.scalar.copy(dst, pt[:])

                for qc in range(RPC // CHUNK):
                    sl = slice(qc * CHUNK, (qc + 1) * CHUNK)
                    proj_into(QT[:, sl], wq, sl)
                for ch in range(N // CHUNK):
                    sl = slice(ch * CHUNK, (ch + 1) * CHUNK)
                    proj_into(KT[:, sl], wk, sl)

            sps = ctx.enter_context(tc.tile_pool(name="sps", bufs=2, space="PSUM"))
            s16p = ctx.enter_context(tc.tile_pool(name="s16p", bufs=2))
            m1p = ctx.enter_context(tc.tile_pool(name="m1p", bufs=2))
            c2p = ctx.enter_context(tc.tile_pool(name="c2p", bufs=2))
            d3p = ctx.enter_context(tc.tile_pool(name="d3p", bufs=2))

            state = {}

            def stage_a(t):
                lhsT = QT[:, t * P:(t + 1) * P]
                pa = sps.tile([P, HALF], f32, tag="sps", name="pa")
                pb = sps.tile([P, HALF], f32, tag="sps", name="pb")
                for h, ps in ((0, pa), (1, pb)):
                    for ch in range(HALF // CHUNK):
                        psl = slice(ch * CHUNK, (ch + 1) * CHUNK)
                        ksl = slice(h * HALF + ch * CHUNK,
                                    h * HALF + (ch + 1) * CHUNK)
                        nc.tensor.matmul(out=ps[:, psl], lhsT=lhsT,
                                         rhs=KT[:, ksl], start=True, stop=True)
                # L1 pairs S[j] with S[j+2048] (pa vs pb).  ACT casts pb fully
                # plus pa's tail to fp16 SBUF (GPSIMD cannot touch PSUM or
                # max; ACT is the PSUM-exit engine); DVE takes the first ZPS
                # columns of pa straight from PSUM (one PSUM operand allowed
                # per DVE instruction) to offload ACT.
                s16b = s16p.tile([P, HALF], f16, tag="s16b", name="s16b")
                nc.scalar.copy(s16b[:], pb[:])
                s16a = s16p.tile([P, HALF - ZPS], f16, tag="s16a", name="s16a")
                nc.scalar.copy(s16a[:], pa[:, ZPS:HALF])
                M1 = m1p.tile([P, HALF], f16, tag="M1", name="M1")
                nc.vector.tensor_tensor(M1[:, 0:ZPS], pa[:, 0:ZPS],
                                        s16b[:, 0:ZPS], op=mx)
                nc.vector.tensor_tensor(M1[:, ZPS:HALF], s16a[:],
                                        s16b[:, ZPS:HALF], op=mx)
                state[t] = M1

            def stage_b(t):
                M1 = state.pop(t)
                C2 = c2p.tile([P, 1024], f16, tag="C2", name="C2")
                nc.vector.tensor_tensor(C2[:], M1[:, 0:1024], M1[:, 1024:2048],
                                        op=mx)
                D3 = d3p.tile([P, NGROUP], f16, tag="D3", name="D3")
                nc.vector.tensor_tensor(D3[:], C2[:, 0:512], C2[:, 512:1024],
                                        op=mx)
                nc.sync.dma_start(d3_d[t * P:(t + 1) * P, :], D3[:])

            for t in range(NT + 1):
                if t < NT:
                    stage_a(t)
                if t >= 1:
                    stage_b(t - 1)

    nc.compile()
    return nc


def _get_program(k=None):
    if "nc" not in _cache:
        _cache["nc"] = _build()
    return _cache["nc"]


def kernel(x, W_q, W_k, top_k):
    import ml_dtypes
    from concourse.bass_utils import run_bass_kernel_spmd

    x = np.asarray(x, dtype=np.float32)
    W_q = np.asarray(W_q, dtype=np.float32)
    W_k = np.asarray(W_k, dtype=np.float32)
    k = int(np.asarray(top_k))
    assert x.shape == (B, N, C) and W_q.shape == (C, DK) and W_k.shape == (C, DK)
    assert 1 <= k <= 8, f"top_k={k} unsupported"

    nc = _get_program()

    scale = np.float32(DK) ** np.float32(-0.5)
    wq16 = np.ascontiguousarray(W_q).astype(ml_dtypes.bfloat16)
    wk16 = np.ascontiguousarray(W_k).astype(ml_dtypes.bfloat16)

    in_maps = []
    for c in range(NCORES):
        b, half = c // 2, c % 2
        # rotate so this core's query columns are [0, RPC); KT/group ids are
        # then in rotated column space and get un-rotated host-side below.
        xT16 = x[b].T.astype(ml_dtypes.bfloat16)
        if half:
            xT16 = np.roll(xT16, -RPC, axis=1)
        in_maps.append({"xk": np.ascontiguousarray(xT16),
                        "wq": wq16, "wk": wk16})

    res = run_bass_kernel_spmd(nc, in_maps, list(range(NCORES)))

    # host refinement: exact fp32 scores for 256 candidate columns per row
    Q = np.matmul(x, W_q)                      # [B, N, DK] fp32
    K = np.matmul(x, W_k)                      # [B, N, DK] fp32

    A = np.zeros((B, N, N), dtype=np.float32)
    m8 = (NGROUP * np.arange(N // NGROUP, dtype=np.int32))[None, None, :]
    for c in range(NCORES):
        b, half = c // 2, c % 2
        D3 = res.results[c]["d3"].astype(np.float32)       # [RPC, 512]
        # top-NCAND groups per row by fp16 group-max code (slack for rounding)
        groups = np.argpartition(-D3, NCAND, axis=1)[:, :NCAND].astype(np.int32)
        cols = (groups[:, :, None] + m8).reshape(RPC, -1)  # [RPC, NCAND*8]
        if half:
            cols = (cols + RPC) % N   # un-rotate candidate column ids
        r0 = half * RPC
        CH = 256
        for rs in range(0, RPC, CH):
            rows = slice(rs, rs + CH)
            ccols = cols[rows]                              # [CH, 256]
            Kc = K[b][ccols]                                # [CH, 256, DK]
            Sc = np.einsum("rd,rcd->rc", Q[b][r0 + rs:r0 + rs + CH], Kc,
                           dtype=np.float32) * scale
            order = np.lexsort((ccols, -Sc), axis=-1)[:, :k]
            topcols = np.take_along_axis(ccols, order, axis=1)
            topS = np.take_along_axis(Sc, order, axis=1)
            mrow = topS.max(axis=1, keepdims=True)
            e = np.exp(topS - mrow)
            vals = (e / e.sum(axis=1, keepdims=True)).astype(np.float32)
            ridx = np.arange(r0 + rs, r0 + rs + CH)[:, None]
            A[b][ridx, topcols] = vals
    return A


# revision 27
# speedup vs baseline: 1.5204x; 1.5204x over previous
"""Trainium2 Bass kernel for DynamicEdgeConstruction (top-k masked softmax
attention matrix).

Computes, for x [B=4, N=4096, C=256], W_q/W_k [256, 64]:
    Q = x @ W_q; K = x @ W_k
    S = Q K^T / sqrt(64)           [B, N, N]
    A = softmax over the top-k entries of each row of S, zeros elsewhere.

Sharding: 8 NeuronCores, 2 per batch element, each handling 2048 query rows
(row-wise sequence parallel; K replicated per batch).

Device algorithm per core (candidate-group formulation): the dense A is
~0.2% nonzero, so the device never materializes it.  Per 128-row tile it
computes S via bf16 PE matmuls (fp32 PSUM) and folds the 4096 columns
through a 2-level max-reduction tree into 1024 fp16 group-maxima (groups
= columns congruent mod 1024), which it ships to the host.  Every true top-k
column's group provably ranks <= 8 among the 512 group-maxima up to
bf16/fp16 rounding, so the host's top-24 group pick has large slack.  The
host gathers the 24x8 candidate columns, recomputes their exact fp32
scores (Q/K host-side), picks the exact top-k with lax.top_k tie
semantics, and scatters the softmax values into the dense fp32 output.

Engine split per tile (ns, cost-model): PE 8 matmuls ~1.7-3.4k; ACT casts
pb + pa-tail to fp16 SBUF ~3.2k; DVE L1 (ZPS cols straight from PSUM) +
L2 + L3 ~3.2k; DMA ships 1 KiB/row of group-maxima.
"""

import numpy as np

B, N, C, DK = 4, 4096, 256, 64
NCORES = 8
RPC = N // 2          # rows per core (2048)
P = 128               # partitions
NT = RPC // P         # row tiles per core (16)
CHUNK = 512           # matmul free-dim chunk (one PSUM bank fp32)
HALF = 2048
NGROUP = 1024         # leaf groups per row (columns congruent mod 1024)
ZPS = 1536            # L1 columns DVE takes straight from PSUM (rest via ACT)
NCAND = 24            # groups the host keeps per row (top-24 of 512)

_cache = {}


def _build():
    import concourse.bass as bass
    import concourse.bacc as bacc
    import concourse.tile as tile
    import concourse.mybir as mybir
    from contextlib import ExitStack

    f32 = mybir.dt.float32
    f16 = mybir.dt.float16
    bf16 = mybir.dt.bfloat16
    u16 = mybir.dt.uint16
    mx = mybir.AluOpType.max

    nc = bacc.Bacc("TRN2", target_bir_lowering=False, debug=False,
                   num_devices=NCORES)

    xk_d = nc.dram_tensor("xk", [C, N], bf16, kind="ExternalInput").ap()
    wq_d = nc.dram_tensor("wq", [C, DK], bf16, kind="ExternalInput").ap()
    wk_d = nc.dram_tensor("wk", [C, DK], bf16, kind="ExternalInput").ap()
    d3_d = nc.dram_tensor("d3", [RPC, NGROUP], f16, kind="ExternalOutput").ap()

    with tile.TileContext(nc) as tc:
        with ExitStack() as ctx:
            const = ctx.enter_context(tc.tile_pool(name="const", bufs=1))

            xk = [const.tile([P, N], bf16, tag=f"xk{i}", name=f"xk{i}")
                  for i in range(2)]
            wq = [const.tile([P, DK], bf16, tag=f"wq{i}", name=f"wq{i}")
                  for i in range(2)]
            wk = [const.tile([P, DK], bf16, tag=f"wk{i}", name=f"wk{i}")
                  for i in range(2)]
            KT = const.tile([DK, N], bf16, tag="KT")
            QT = const.tile([DK, RPC], bf16, tag="QT")

            # weights first (small), then x in column chunks on two queues
            nc.gpsimd.dma_start(wk[0][:], wk_d[0:P, :])
            nc.gpsimd.dma_start(wk[1][:], wk_d[P:2 * P, :])
            nc.gpsimd.dma_start(wq[0][:], wq_d[0:P, :])
            nc.gpsimd.dma_start(wq[1][:], wq_d[P:2 * P, :])
            XCH = 1024
            for chx in range(N // XCH):
                sl = slice(chx * XCH, (chx + 1) * XCH)
                nc.sync.dma_start(xk[0][:, sl], xk_d[0:P, sl])
                nc.scalar.dma_start(xk[1][:, sl], xk_d[P:2 * P, sl])

            # Projections: KT = wk^T @ x (contraction over C in two halves),
            # QT from columns [0, RPC) -- the host rotates x per core so this
            # core's query columns always sit first, and un-rotates the
            # candidate column ids afterwards.
            with tc.tile_pool(name="proj_ps", bufs=4, space="PSUM") as proj_ps:
                def proj_into(dst, w, sl, i):
                    pt = proj_ps.tile([DK, CHUNK], f32, tag="proj", name="pt")
                    nc.tensor.matmul(out=pt[:], lhsT=w[0][:],
                                     rhs=xk[0][:, sl], start=True, stop=False)
                    nc.tensor.matmul(out=pt[:], lhsT=w[1][:],
                                     rhs=xk[1][:, sl], start=False, stop=True)
                    # alternate the PSUM-drain between ACT and DVE
                    if i % 2 == 0:
                        nc.scalar.copy(dst, pt[:])
                    else:
                        nc.vector.tensor_scalar_mul(dst, pt[:], 1.0)

                # KT first: the main loop needs all of KT but only QT chunk 0
                # to start; later QT chunks land during the first tiles.
                for ch in range(N // CHUNK):
                    sl = slice(ch * CHUNK, (ch + 1) * CHUNK)
                    proj_into(KT[:, sl], wk, sl, ch)
                for qc in range(RPC // CHUNK):
                    sl = slice(qc * CHUNK, (qc + 1) * CHUNK)
                    proj_into(QT[:, sl], wq, sl, qc)

            sps = ctx.enter_context(tc.tile_pool(name="sps", bufs=1, space="PSUM"))
            s16p = ctx.enter_context(tc.tile_pool(name="s16p", bufs=2))
            m1p = ctx.enter_context(tc.tile_pool(name="m1p", bufs=2))
            c2p = ctx.enter_context(tc.tile_pool(name="c2p", bufs=2))
            d3p = ctx.enter_context(tc.tile_pool(name="d3p", bufs=2))

            state = {}

            def stage_a(t):
                lhsT = QT[:, t * P:(t + 1) * P]
                # S lands in four separate PSUM tiles (paL paR pbP pbT) so a
                # next-tile matmul only waits on that tile's own consumer
                # (dependency tracking is per-tile).  ACT casts paL/paR/pbT to
                # fp16 SBUF (GPSIMD cannot touch PSUM or max); DVE's L1 reads
                # pbP straight from PSUM (one PSUM operand per instruction).
                paL = sps.tile([P, 1024], f32, tag="paL", name="paL")
                paR = sps.tile([P, 1024], f32, tag="paR", name="paR")
                pbP = sps.tile([P, ZPS], f32, tag="pbP", name="pbP")
                pbT = sps.tile([P, HALF - ZPS], f32, tag="pbT", name="pbT")
                for ps, base, w in ((paL, 0, 1024), (paR, 1024, 1024),
                                    (pbP, HALF, ZPS), (pbT, HALF + ZPS, HALF - ZPS)):
                    for ch in range(w // CHUNK):
                        psl = slice(ch * CHUNK, (ch + 1) * CHUNK)
                        ksl = slice(base + ch * CHUNK, base + (ch + 1) * CHUNK)
                        nc.tensor.matmul(out=ps[:, psl], lhsT=lhsT,
                                         rhs=KT[:, ksl], start=True, stop=True)
                s16a = s16p.tile([P, HALF], f16, tag="s16a", name="s16a")
                nc.scalar.copy(s16a[:, 0:1024], paL[:])
                nc.scalar.copy(s16a[:, 1024:HALF], paR[:])
                s16b = s16p.tile([P, HALF - ZPS], f16, tag="s16b", name="s16b")
                nc.scalar.copy(s16b[:], pbT[:])
                state[t] = (pbP, s16a, s16b)

            def stage_b(t):
                pbP, s16a, s16b = state.pop(t)
                # L1 pairs S[j] (s16a) with S[j+2048] (pbP / s16b)
                M1 = m1p.tile([P, HALF], f16, tag="M1", name="M1")
                nc.vector.tensor_tensor(M1[:, 0:ZPS], pbP[:],
                                        s16a[:, 0:ZPS], op=mx)
                nc.vector.tensor_tensor(M1[:, ZPS:HALF], s16b[:],
                                        s16a[:, ZPS:HALF], op=mx)
                C2 = c2p.tile([P, NGROUP], f16, tag="C2", name="C2")
                nc.vector.tensor_tensor(C2[:], M1[:, 0:1024], M1[:, 1024:2048],
                                        op=mx)
                nc.sync.dma_start(d3_d[t * P:(t + 1) * P, :], C2[:])

            for t in range(NT + 1):
                if t < NT:
                    stage_a(t)
                if t >= 1:
                    stage_b(t - 1)

    nc.compile()
    return nc


def _get_program(k=None):
    if "nc" not in _cache:
        _cache["nc"] = _build()
    return _cache["nc"]


def kernel(x, W_q, W_k, top_k):
    import ml_dtypes
    from concourse.bass_utils import run_bass_kernel_spmd

    x = np.asarray(x, dtype=np.float32)
    W_q = np.asarray(W_q, dtype=np.float32)
    W_k = np.asarray(W_k, dtype=np.float32)
    k = int(np.asarray(top_k))
    assert x.shape == (B, N, C) and W_q.shape == (C, DK) and W_k.shape == (C, DK)
    assert 1 <= k <= 8, f"top_k={k} unsupported"

    nc = _get_program()

    scale = np.float32(DK) ** np.float32(-0.5)
    wq16 = np.ascontiguousarray(W_q).astype(ml_dtypes.bfloat16)
    wk16 = np.ascontiguousarray(W_k).astype(ml_dtypes.bfloat16)

    in_maps = []
    for c in range(NCORES):
        b, half = c // 2, c % 2
        # rotate so this core's query columns are [0, RPC); KT/group ids are
        # then in rotated column space and get un-rotated host-side below.
        xT16 = x[b].T.astype(ml_dtypes.bfloat16)
        if half:
            xT16 = np.roll(xT16, -RPC, axis=1)
        in_maps.append({"xk": np.ascontiguousarray(xT16),
                        "wq": wq16, "wk": wk16})

    res = run_bass_kernel_spmd(nc, in_maps, list(range(NCORES)))

    # host refinement: exact fp32 scores for 256 candidate columns per row
    Q = np.matmul(x, W_q)                      # [B, N, DK] fp32
    K = np.matmul(x, W_k)                      # [B, N, DK] fp32

    A = np.zeros((B, N, N), dtype=np.float32)
    m8 = (NGROUP * np.arange(N // NGROUP, dtype=np.int32))[None, None, :]
    for c in range(NCORES):
        b, half = c // 2, c % 2
        D3 = res.results[c]["d3"].astype(np.float32)       # [RPC, 512]
        # top-NCAND groups per row by fp16 group-max code (slack for rounding)
        groups = np.argpartition(-D3, NCAND, axis=1)[:, :NCAND].astype(np.int32)
        cols = (groups[:, :, None] + m8).reshape(RPC, -1)  # [RPC, NCAND*8]
        if half:
            cols = (cols + RPC) % N   # un-rotate candidate column ids
        r0 = half * RPC
        CH = 256
        for rs in range(0, RPC, CH):
            rows = slice(rs, rs + CH)
            ccols = cols[rows]                              # [CH, 256]
            Kc = K[b][ccols]                                # [CH, 256, DK]
            Sc = np.einsum("rd,rcd->rc", Q[b][r0 + rs:r0 + rs + CH], Kc,
                           dtype=np.float32) * scale
            order = np.lexsort((ccols, -Sc), axis=-1)[:, :k]
            topcols = np.take_along_axis(ccols, order, axis=1)
            topS = np.take_along_axis(Sc, order, axis=1)
            mrow = topS.max(axis=1, keepdims=True)
            e = np.exp(topS - mrow)
            vals = (e / e.sum(axis=1, keepdims=True)).astype(np.float32)
            ridx = np.arange(r0 + rs, r0 + rs + CH)[:, None]
            A[b][ridx, topcols] = vals
    return A


# revision 33
# speedup vs baseline: 1.6796x; 1.1048x over previous
"""Trainium2 Bass kernel for DynamicEdgeConstruction (top-k masked softmax
attention matrix).

Computes, for x [B=4, N=4096, C=256], W_q/W_k [256, 64]:
    Q = x @ W_q; K = x @ W_k
    S = Q K^T / sqrt(64)           [B, N, N]
    A = softmax over the top-k entries of each row of S, zeros elsewhere.

Sharding: 8 NeuronCores, 2 per batch element, each handling 2048 query rows
(row-wise sequence parallel; K replicated per batch).

Device algorithm per core (candidate-group formulation): the dense A is
~0.2% nonzero, so the device never materializes it.  Per 128-row tile it
computes S via bf16 PE matmuls (fp32 PSUM) and folds the 4096 columns
through a 2-level max-reduction tree into 1024 fp16 group-maxima (groups
= columns congruent mod 1024), which it ships to the host.  Every true top-k
column's group provably ranks <= 8 among the 512 group-maxima up to
bf16/fp16 rounding, so the host's top-24 group pick has large slack.  The
host gathers the 24x8 candidate columns, recomputes their exact fp32
scores (Q/K host-side), picks the exact top-k with lax.top_k tie
semantics, and scatters the softmax values into the dense fp32 output.

Engine split per tile (ns, cost-model): PE 8 matmuls ~1.7-3.4k; ACT casts
pb + pa-tail to fp16 SBUF ~3.2k; DVE L1 (ZPS cols straight from PSUM) +
L2 + L3 ~3.2k; DMA ships 1 KiB/row of group-maxima.
"""

import numpy as np

B, N, C, DK = 4, 4096, 256, 64
NCORES = 8
RPC = N // 2          # rows per core (2048)
P = 128               # partitions
NT = RPC // P         # row tiles per core (16)
CHUNK = 512           # matmul free-dim chunk (one PSUM bank fp32)
HALF = 2048
NGROUP = 2048         # leaf groups per row (columns congruent mod 2048)
ZPS = 1536            # L1 columns DVE takes straight from PSUM (rest via ACT)
NCAND = 24            # groups the host keeps per row (top-24 of 512)

_cache = {}


def _build():
    import concourse.bass as bass
    import concourse.bacc as bacc
    import concourse.tile as tile
    import concourse.mybir as mybir
    from contextlib import ExitStack

    f32 = mybir.dt.float32
    f16 = mybir.dt.float16
    bf16 = mybir.dt.bfloat16
    u16 = mybir.dt.uint16
    mx = mybir.AluOpType.max

    nc = bacc.Bacc("TRN2", target_bir_lowering=False, debug=False,
                   num_devices=NCORES)

    xk_d = nc.dram_tensor("xk", [C, N], bf16, kind="ExternalInput").ap()
    wq_d = nc.dram_tensor("wq", [C, DK], bf16, kind="ExternalInput").ap()
    wk_d = nc.dram_tensor("wk", [C, DK], bf16, kind="ExternalInput").ap()
    d3_d = nc.dram_tensor("d3", [RPC, NGROUP], f16, kind="ExternalOutput").ap()

    with tile.TileContext(nc) as tc:
        with ExitStack() as ctx:
            const = ctx.enter_context(tc.tile_pool(name="const", bufs=1))

            xk = [const.tile([P, N], bf16, tag=f"xk{i}", name=f"xk{i}")
                  for i in range(2)]
            wq = [const.tile([P, DK], bf16, tag=f"wq{i}", name=f"wq{i}")
                  for i in range(2)]
            wk = [const.tile([P, DK], bf16, tag=f"wk{i}", name=f"wk{i}")
                  for i in range(2)]
            # KT/QT as per-1024/512-col tiles: dependency tracking is
            # per-tile, so S matmuls only wait for the chunks they read.
            KTt = [const.tile([DK, 1024], bf16, tag=f"KT{i}", name=f"KT{i}")
                   for i in range(4)]
            QTt = [const.tile([DK, CHUNK], bf16, tag=f"QT{i}", name=f"QT{i}")
                   for i in range(4)]

            # weights first (small), then x in column chunks on two queues
            nc.gpsimd.dma_start(wk[0][:], wk_d[0:P, :])
            nc.gpsimd.dma_start(wk[1][:], wk_d[P:2 * P, :])
            nc.gpsimd.dma_start(wq[0][:], wq_d[0:P, :])
            nc.gpsimd.dma_start(wq[1][:], wq_d[P:2 * P, :])
            XCH = 1024
            for chx in range(N // XCH):
                sl = slice(chx * XCH, (chx + 1) * XCH)
                nc.sync.dma_start(xk[0][:, sl], xk_d[0:P, sl])
                nc.scalar.dma_start(xk[1][:, sl], xk_d[P:2 * P, sl])

            # Projections: KT = wk^T @ x (contraction over C in two halves),
            # QT from columns [0, RPC) -- the host rotates x per core so this
            # core's query columns always sit first, and un-rotates the
            # candidate column ids afterwards.
            with tc.tile_pool(name="proj_ps", bufs=4, space="PSUM") as proj_ps:
                def proj_into(dst, w, sl, i):
                    pt = proj_ps.tile([DK, CHUNK], f32, tag="proj", name="pt")
                    nc.tensor.matmul(out=pt[:], lhsT=w[0][:],
                                     rhs=xk[0][:, sl], start=True, stop=False)
                    nc.tensor.matmul(out=pt[:], lhsT=w[1][:],
                                     rhs=xk[1][:, sl], start=False, stop=True)
                    # alternate the PSUM-drain between ACT and DVE
                    if i % 2 == 0:
                        nc.scalar.copy(dst, pt[:])
                    else:
                        nc.vector.tensor_scalar_mul(dst, pt[:], 1.0)

                for ch in range(N // CHUNK):
                    sl = slice(ch * CHUNK, (ch + 1) * CHUNK)
                    proj_into(KTt[ch // 2][:, (ch % 2) * CHUNK:
                                           (ch % 2 + 1) * CHUNK], wk, sl, ch)
                for qc in range(RPC // CHUNK):
                    qsl = slice(qc * CHUNK, (qc + 1) * CHUNK)
                    proj_into(QTt[qc][:, :], wq, qsl, qc)

            sps = ctx.enter_context(tc.tile_pool(name="sps", bufs=1, space="PSUM"))
            s16p = ctx.enter_context(tc.tile_pool(name="s16p", bufs=2))
            m1p = ctx.enter_context(tc.tile_pool(name="m1p", bufs=4))
            c2p = ctx.enter_context(tc.tile_pool(name="c2p", bufs=2))
            d3p = ctx.enter_context(tc.tile_pool(name="d3p", bufs=2))

            state = {}

            def stage_a(t):
                qt = QTt[t // 4]
                lhsT = qt[:, (t % 4) * P:(t % 4 + 1) * P]
                # S lands in four separate PSUM tiles (paL paR pbP pbT) so a
                # next-tile matmul only waits on that tile's own consumer
                # (dependency tracking is per-tile).  ACT casts paL/paR/pbT to
                # fp16 SBUF (GPSIMD cannot touch PSUM or max); DVE's L1 reads
                # pbP straight from PSUM (one PSUM operand per instruction).
                paL = sps.tile([P, 1024], f32, tag="paL", name="paL")
                paR = sps.tile([P, 1024], f32, tag="paR", name="paR")
                pbL = sps.tile([P, 1024], f32, tag="pbL", name="pbL")
                pbR = sps.tile([P, 1024], f32, tag="pbR", name="pbR")
                for ps, kt in ((paL, KTt[0]), (paR, KTt[1]),
                               (pbL, KTt[2]), (pbR, KTt[3])):
                    for ch in range(2):
                        psl = slice(ch * CHUNK, (ch + 1) * CHUNK)
                        nc.tensor.matmul(out=ps[:, psl], lhsT=lhsT,
                                         rhs=kt[:, psl], start=True, stop=True)
                s16a = s16p.tile([P, HALF], f16, tag="s16a", name="s16a")
                nc.scalar.copy(s16a[:, 0:1024], paL[:])
                nc.scalar.copy(s16a[:, 1024:HALF], paR[:])
                state[t] = (pbL, pbR, s16a)

            def stage_b(t):
                pbL, pbR, s16a = state.pop(t)
                # L1 pairs S[j] (s16a) with S[j+2048] (pbL / pbR)
                M1 = m1p.tile([P, HALF], f16, tag="M1", name="M1")
                nc.vector.tensor_tensor(M1[:, 0:1024], pbL[:],
                                        s16a[:, 0:1024], op=mx)
                nc.vector.tensor_tensor(M1[:, 1024:HALF], pbR[:],
                                        s16a[:, 1024:HALF], op=mx)
                nc.sync.dma_start(d3_d[t * P:(t + 1) * P, :], M1[:])

            for t in range(NT + 1):
                if t < NT:
                    stage_a(t)
                if t >= 1:
                    stage_b(t - 1)

    nc.compile()
    return nc


def _get_program(k=None):
    if "nc" not in _cache:
        _cache["nc"] = _build()
    return _cache["nc"]


def kernel(x, W_q, W_k, top_k):
    import ml_dtypes
    from concourse.bass_utils import run_bass_kernel_spmd

    x = np.asarray(x, dtype=np.float32)
    W_q = np.asarray(W_q, dtype=np.float32)
    W_k = np.asarray(W_k, dtype=np.float32)
    k = int(np.asarray(top_k))
    assert x.shape == (B, N, C) and W_q.shape == (C, DK) and W_k.shape == (C, DK)
    assert 1 <= k <= 8, f"top_k={k} unsupported"

    nc = _get_program()

    scale = np.float32(DK) ** np.float32(-0.5)
    wq16 = np.ascontiguousarray(W_q).astype(ml_dtypes.bfloat16)
    wk16 = np.ascontiguousarray(W_k).astype(ml_dtypes.bfloat16)

    in_maps = []
    for c in range(NCORES):
        b, half = c // 2, c % 2
        # rotate so this core's query columns are [0, RPC); KT/group ids are
        # then in rotated column space and get un-rotated host-side below.
        xT16 = x[b].T.astype(ml_dtypes.bfloat16)
        if half:
            xT16 = np.roll(xT16, -RPC, axis=1)
        in_maps.append({"xk": np.ascontiguousarray(xT16),
                        "wq": wq16, "wk": wk16})

    res = run_bass_kernel_spmd(nc, in_maps, list(range(NCORES)))

    # host refinement: exact fp32 scores for 256 candidate columns per row
    Q = np.matmul(x, W_q)                      # [B, N, DK] fp32
    K = np.matmul(x, W_k)                      # [B, N, DK] fp32

    A = np.zeros((B, N, N), dtype=np.float32)
    m8 = (NGROUP * np.arange(N // NGROUP, dtype=np.int32))[None, None, :]
    for c in range(NCORES):
        b, half = c // 2, c % 2
        D3 = res.results[c]["d3"].astype(np.float32)       # [RPC, 512]
        # top-NCAND groups per row by fp16 group-max code (slack for rounding)
        groups = np.argpartition(-D3, NCAND, axis=1)[:, :NCAND].astype(np.int32)
        cols = (groups[:, :, None] + m8).reshape(RPC, -1)  # [RPC, NCAND*8]
        if half:
            cols = (cols + RPC) % N   # un-rotate candidate column ids
        r0 = half * RPC
        CH = 256
        for rs in range(0, RPC, CH):
            rows = slice(rs, rs + CH)
            ccols = cols[rows]                              # [CH, 256]
            Kc = K[b][ccols]                                # [CH, 256, DK]
            Sc = np.einsum("rd,rcd->rc", Q[b][r0 + rs:r0 + rs + CH], Kc,
                           dtype=np.float32) * scale
            order = np.lexsort((ccols, -Sc), axis=-1)[:, :k]
            topcols = np.take_along_axis(ccols, order, axis=1)
            topS = np.take_along_axis(Sc, order, axis=1)
            mrow = topS.max(axis=1, keepdims=True)
            e = np.exp(topS - mrow)
            vals = (e / e.sum(axis=1, keepdims=True)).astype(np.float32)
            ridx = np.arange(r0 + rs, r0 + rs + CH)[:, None]
            A[b][ridx, topcols] = vals
    return A
